# revision 1
# baseline (speedup 1.0000x reference)
"""GNN message-passing aggregator on 8 Trainium2 NeuronCores.

  h = relu(relu(z @ U1 + c1) @ U2 + c2)
  z = segment_sum(relu(relu(y[src] @ W1 + b1) @ W2 + b2), dst)

Strategy:
  * MLP(y[src]) == MLP(y)[src]: compute the pre-MLP once per node (m), then the
    per-edge work collapses to gather m[src] + segment-sum by dst.
  * Edges are sharded by dst ownership (6250 nodes per core) on the host, so the
    segment-sum is core-local: no collectives at all.
  * Per core the dst nodes form 49 windows of 128; a window's edges are packed
    into 128-slot columns.  Each column is gathered from the m table in HBM via
    dma_gather (int16 indices -> the table is addressed as lo/hi halves) and
    accumulated into the window's PSUM tile with a matmul whose stationary
    operand is a selection matrix P[p, n] = (nodeid[p] == n), built on the DVE
    from a host-streamed compact nodeid vector.  PSUM accumulation gives an
    exact fp32 segment sum.  The gather (GPSIMD Q7 descriptor generation,
    ~8.4 ns/slot) is the hard floor of this kernel; everything else overlaps it.
  * Phase 1 (m) runs before the gathers that need it: lo-half gathers only wait
    on the lo half of the m table (junction NOP deps), hi on the rest.
  * Phase 3 (update MLP) is tiled in groups of 4 windows so it pipelines into
    the tail of the gather stream.
"""

import os

import numpy as np

# ---------------------------------------------------------------- constants
N_NODES = 50000
D = 64
NC = 8                      # cores
W = 128                     # window size == psum partitions
SPLIT = 32768               # int16-addressable half of the m table
OP_COLS = 8                 # gather columns per dma_gather op (8*128=1024 idx)
CHUNK = 512                 # dense-MLP T-major matmul chunk
GRP = 4                     # windows per phase-3 group (GRP*W == CHUNK)

_COMPILED = {}


def _dims():
    npc = N_NODES // NC
    nw = (npc + W - 1) // W
    npad = nw * W
    mt_rows = ((N_NODES + 127) // 128) * 128
    return npc, nw, npad, mt_rows


# ------------------------------------------------------------ host schedule
def _host_schedule(src, dst):
    """Shard edges by dst ownership, balance nodes into windows, pack columns.

    Returns (sched, per_core): sched is the shared compile-time schedule
    (identical across cores); per_core holds the input arrays per core.
    """
    NPC, NW, NPAD, _ = _dims()

    percore_groups = []
    percore_perm = []
    clo = np.zeros((NC, NW), np.int64)
    chi = np.zeros((NC, NW), np.int64)

    for c in range(NC):
        lo_n, hi_n = c * NPC, (c + 1) * NPC
        sel = (dst >= lo_n) & (dst < hi_n)
        s = src[sel].astype(np.int64)
        d = (dst[sel] - lo_n).astype(np.int64)
        deg = np.bincount(d, minlength=NPC)

        # balance nodes into NW windows by degree (greedy, descending)
        order = np.argsort(-deg, kind="stable")
        wload = np.zeros(NW, np.int64)
        wcount = np.zeros(NW, np.int64)
        assign = np.zeros(NPC, np.int64)
        label = np.zeros(NPC, np.int64)
        for n in order:
            wavail = np.flatnonzero(wcount < W)
            wsel = wavail[np.argmin(wload[wavail])]
            assign[n] = wsel
            label[n] = wcount[wsel]
            wcount[wsel] += 1
            wload[wsel] += deg[n]

        ew = assign[d]
        is_lo = s < SPLIT
        groups = {}
        for wdw in range(NW):
            m_w = ew == wdw
            for t in range(2):
                m_t = m_w & (is_lo if t == 0 else ~is_lo)
                es = s[m_t]
                el = label[d[m_t]]
                o = np.argsort(es, kind="stable")   # src-sorted for HBM locality
                groups[(wdw, t)] = (es[o], el[o])
                cnt = (len(es) + W - 1) // W
                if t == 0:
                    clo[c, wdw] = cnt
                else:
                    chi[c, wdw] = cnt
        percore_groups.append(groups)
        perm = np.full(NPAD, -1, np.int64)
        perm[assign * W + label] = np.arange(NPC) + lo_n
        percore_perm.append(perm)

    CLo = np.maximum(clo.max(0), 1)
    CHi = np.maximum(chi.max(0), 1)
    lo_off = np.concatenate([[0], np.cumsum(CLo)])
    hi_off = np.concatenate([[0], np.cumsum(CHi)])
    n_lo, n_hi = int(lo_off[-1]), int(hi_off[-1])

    per_core = []
    for c in range(NC):
        groups = percore_groups[c]
        idx = [np.zeros(n_lo * W, np.int16), np.zeros(n_hi * W, np.int16)]
        nid = [np.full(n_lo * W, -1.0, np.float32),
               np.full(n_hi * W, -1.0, np.float32)]
        for wdw in range(NW):
            for t, (cnt, off) in enumerate(((CLo, lo_off), (CHi, hi_off))):
                es, el = groups[(wdw, t)]
                base = int(off[wdw]) * W
                k = len(es)
                idx[t][base:base + k] = (es if t == 0 else es - SPLIT).astype(np.int16)
                nid[t][base:base + k] = el.astype(np.float32)
        # wrap idx into per-op [16, ni/16] layout replicated to 128 partitions
        wrapped = []
        for t, ncols in ((0, n_lo), (1, n_hi)):
            flat = idx[t]
            pos, blocks = 0, []
            while pos < ncols:
                k = min(OP_COLS, ncols - pos)
                ni = k * W
                op = flat[pos * W:(pos + k) * W]
                blk = op.reshape(ni // 16, 16).T            # [16, ni/16]
                blocks.append(np.tile(blk, (8, 1)))          # [128, ni/16]
                pos += k
            wrapped.append(np.concatenate(blocks, axis=1) if blocks
                           else np.zeros((128, 8), np.int16))
        nodeid_2d = np.concatenate([nid[0], nid[1]]).reshape(n_lo + n_hi, W).T
        per_core.append({
            "idx_lo": np.ascontiguousarray(wrapped[0]),
            "idx_hi": np.ascontiguousarray(wrapped[1]),
            "nodeid": np.ascontiguousarray(nodeid_2d.astype(np.float32)),
            "perm": percore_perm[c],
        })

    sched = {"CLo": CLo.astype(int).tolist(), "CHi": CHi.astype(int).tolist(),
             "n_lo": n_lo, "n_hi": n_hi}
    return sched, per_core


# ------------------------------------------------------------- bass program
def _build_program(sched):
    import concourse.bacc as bacc
    import concourse.mybir as mybir
    import concourse.tile as tile
    from concourse.masks import make_identity
    from concourse.tile import add_dep_helper

    f32 = mybir.dt.float32
    i16 = mybir.dt.int16
    Relu = mybir.ActivationFunctionType.Relu

    NPC, NW, NPAD, MT_ROWS = _dims()
    CLo, CHi = sched["CLo"], sched["CHi"]
    n_lo, n_hi = sched["n_lo"], sched["n_hi"]
    n_cols = n_lo + n_hi
    NCH = (MT_ROWS + CHUNK - 1) // CHUNK
    LO_CH = SPLIT // CHUNK          # chunks 0..LO_CH-1 cover m rows < SPLIT

    nc = bacc.Bacc()
    yT_in = nc.dram_tensor("yT", [D + 1, MT_ROWS], f32, kind="ExternalInput")
    wb1_in = nc.dram_tensor("wb1", [D + 1, D], f32, kind="ExternalInput")
    wb2_in = nc.dram_tensor("wb2", [D + 1, D], f32, kind="ExternalInput")
    ub1_in = nc.dram_tensor("ub1", [D + 1, D], f32, kind="ExternalInput")
    ub2_in = nc.dram_tensor("ub2", [D + 1, D], f32, kind="ExternalInput")
    idxlo_in = nc.dram_tensor("idx_lo", [128, n_lo * 8], i16, kind="ExternalInput")
    idxhi_in = nc.dram_tensor("idx_hi", [128, n_hi * 8], i16, kind="ExternalInput")
    nodeid_in = nc.dram_tensor("nodeid", [128, n_cols], f32, kind="ExternalInput")
    iota_in = nc.dram_tensor("iota128", [128, 128], f32, kind="ExternalInput")
    m_dram = nc.dram_tensor("m_scratch", [MT_ROWS, D], f32, kind="Internal")
    h_out = nc.dram_tensor("h_out", [NPAD, D], f32, kind="ExternalOutput")
    debug = bool(int(os.environ.get("KERNEL_DEBUG_Z", "0")))
    if debug:
        z_out = nc.dram_tensor("z_out", [NPAD, D], f32, kind="ExternalOutput")
        m_out = nc.dram_tensor("m_out", [MT_ROWS, D], f32, kind="ExternalOutput")

    with tile.TileContext(nc) as tc:
        with tc.tile_pool(name="const", bufs=1) as cpool, \
             tc.tile_pool(name="idxp", bufs=1) as idxp, \
             tc.tile_pool(name="zpool", bufs=1) as zpool:
            wb1 = cpool.tile([D + 1, D], f32, tag="wb1")
            wb2 = cpool.tile([D + 1, D], f32, tag="wb2")
            ub1 = cpool.tile([D + 1, D], f32, tag="ub1")
            ub2 = cpool.tile([D + 1, D], f32, tag="ub2")
            iota = cpool.tile([128, 128], f32, tag="iota")
            ident = cpool.tile([128, 128], f32, tag="ident")
            nc.sync.dma_start(out=wb1[:], in_=wb1_in[:])
            nc.sync.dma_start(out=wb2[:], in_=wb2_in[:])
            nc.sync.dma_start(out=ub1[:], in_=ub1_in[:])
            nc.sync.dma_start(out=ub2[:], in_=ub2_in[:])
            nc.sync.dma_start(out=iota[:], in_=iota_in[:])
            make_identity(nc, ident[:])

            # phase-2 static inputs: load up front (independent of phase 1)
            idx_lo_t = idxp.tile([128, n_lo * 8], i16, tag="ilo")
            idx_hi_t = idxp.tile([128, n_hi * 8], i16, tag="ihi")
            idx_t = [idx_lo_t, idx_hi_t]
            nc.sync.dma_start(out=idx_lo_t[:], in_=idxlo_in[:])
            nc.sync.dma_start(out=idx_hi_t[:], in_=idxhi_in[:])
            nodeid_t = idxp.tile([128, n_cols], f32, tag="nid")
            nc.sync.dma_start(out=nodeid_t[:], in_=nodeid_in[:])

            # ------------ phase 1: m = relu(relu(y@W1+b1)@W2+b2) -> m_dram ---
            m_writes = []        # per-chunk m-write DMA instructions
            with tc.tile_pool(name="p1y", bufs=3) as p1y, \
                 tc.tile_pool(name="p1h", bufs=1) as p1h, \
                 tc.tile_pool(name="p1m", bufs=3) as p1m, \
                 tc.tile_pool(name="p1ps", bufs=2, space="PSUM") as p1ps, \
                 tc.tile_pool(name="p1ps2", bufs=3, space="PSUM") as p1ps2:
                # two persistent h1 buffers with the ones-row preset ONCE, so
                # the per-chunk ACT(relu) does not serialize against a memset.
                h1a = p1h.tile([D + 1, CHUNK], f32, tag="h1a")
                h1b = p1h.tile([D + 1, CHUNK], f32, tag="h1b")
                nc.gpsimd.memset(h1a[D:D + 1, :], 1.0)
                nc.gpsimd.memset(h1b[D:D + 1, :], 1.0)
                h1bufs = [h1a, h1b]
                for ch in range(NCH):
                    c0 = ch * CHUNK
                    cw = min(CHUNK, MT_ROWS - c0)
                    ytile = p1y.tile([D + 1, CHUNK], f32, tag="ytile")
                    nc.sync.dma_start(out=ytile[:, :cw], in_=yT_in[:, c0:c0 + cw])
                    ps = p1ps.tile([D, CHUNK], f32, tag="ps1")
                    nc.tensor.matmul(out=ps[:, :cw], lhsT=wb1[:], rhs=ytile[:, :cw],
                                     start=True, stop=True)
                    h1c = h1bufs[ch % 2]
                    nc.scalar.activation(out=h1c[:D, :cw], in_=ps[:, :cw], func=Relu)
                    mch = p1m.tile([128, (CHUNK // 128) * D], f32, tag="mch")
                    for i in range(cw // 128):
                        ps2 = p1ps2.tile([128, D], f32, tag="ps2")
                        nc.tensor.matmul(out=ps2[:],
                                         lhsT=h1c[:, i * 128:(i + 1) * 128],
                                         rhs=wb2[:], start=True, stop=True)
                        nc.vector.tensor_scalar_max(
                            out=mch[:, i * D:(i + 1) * D], in0=ps2[:], scalar1=0.0)
                    wri = nc.scalar.dma_start(
                        out=m_dram[c0:c0 + cw, :].rearrange(
                            "(t p) d -> p t d", p=128),
                        in_=mch[:, :(cw // 128) * D].rearrange(
                            "p (t d) -> p t d", d=D))
                    m_writes.append(wri)

            # junction NOPs: gathers of each table half wait only on the
            # m-writes covering that half (Tile does not track DRAM RAW deps).
            jlo = nc.sync.nop(nofuse=True)
            jhi = nc.sync.nop(nofuse=True)
            for ch, wri in enumerate(m_writes):
                tgt = jlo if ch < LO_CH else jhi
                add_dep_helper(tgt.ins, wri.ins, sync=True,
                               reason="m table half complete")
            # hi junction also needs... (hi gathers only touch rows >= SPLIT)
            junction = [jlo, jhi]

            # ------------ phase 2: gather + segment-sum ----------------------
            z_grps = []
            NGRP = (NW + GRP - 1) // GRP
            with tc.tile_pool(name="gpool", bufs=6) as gpool, \
                 tc.tile_pool(name="ppool", bufs=4) as ppool, \
                 tc.tile_pool(name="zg", bufs=NGRP) as zgp, \
                 tc.tile_pool(name="wps", bufs=4, space="PSUM") as wps:
                tables = [m_dram[0:SPLIT, :], m_dram[SPLIT:MT_ROWS, :]]
                ncols_t = [n_lo, n_hi]
                g_tiles = [{}, {}]

                def ensure_op(t, col):
                    o = col // OP_COLS
                    if o in g_tiles[t]:
                        return g_tiles[t][o]
                    k = min(OP_COLS, ncols_t[t] - o * OP_COLS)
                    g = gpool.tile([128, k, D], f32, tag=f"g{t}")
                    ni = k * W
                    gi = nc.gpsimd.dma_gather(
                        out_ap=g[:], in_ap=tables[t],
                        idxs_ap=idx_t[t][:, o * OP_COLS * 8:o * OP_COLS * 8 + k * 8],
                        num_idxs=ni, num_idxs_reg=ni, elem_size=D)
                    add_dep_helper(gi.ins, junction[t].ins, sync=True,
                                   reason="gather after m half ready")
                    g_tiles[t][o] = g
                    return g

                lo_base, hi_base = 0, 0
                zgrp = None
                for wdw in range(NW):
                    if wdw % GRP == 0:
                        gw = min(GRP, NW - wdw)
                        zgrp = zgp.tile([128, gw * D], f32, tag="zgt")
                        z_grps.append(zgrp)
                    zw = wps.tile([128, D], f32, tag="zw")
                    total = CLo[wdw] + CHi[wdw]
                    ci = 0
                    for t, cnt, base in ((0, CLo[wdw], lo_base),
                                         (1, CHi[wdw], hi_base)):
                        for j in range(cnt):
                            col = base + j
                            g = ensure_op(t, col)
                            sub = col - (col // OP_COLS) * OP_COLS
                            gcol = col if t == 0 else n_lo + col
                            P = ppool.tile([128, 128], f32, tag="P")
                            nc.vector.tensor_tensor(
                                out=P[:],
                                in0=nodeid_t[:, gcol:gcol + 1].to_broadcast(
                                    [128, 128]),
                                in1=iota[:], op=mybir.AluOpType.is_equal)
                            nc.tensor.matmul(out=zw[:], lhsT=P[:],
                                             rhs=g[:, sub, :],
                                             start=(ci == 0),
                                             stop=(ci == total - 1))
                            ci += 1
                    lo_base += CLo[wdw]
                    hi_base += CHi[wdw]
                    nc.scalar.copy(out=zgrp[:, (wdw % GRP) * D:(wdw % GRP + 1) * D],
                                   in_=zw[:])

                if debug:
                    for gi_, zgrp_ in enumerate(z_grps):
                        gw = zgrp_.shape[1] // D
                        nc.sync.dma_start(
                            out=z_out[gi_ * GRP * 128:(gi_ * GRP + gw) * 128, :]
                                .rearrange("(t p) d -> p t d", p=128),
                            in_=zgrp_[:].rearrange("p (t d) -> p t d", d=D))
                    mo = nc.sync.dma_start(out=m_out[:], in_=m_dram[:])
                    add_dep_helper(mo.ins, junction[0].ins, sync=True, reason="dbg")
                    add_dep_helper(mo.ins, junction[1].ins, sync=True, reason="dbg")

                # ------------ phase 3: h = relu(relu(z@U1+c1)@U2+c2) ---------
                # grouped by GRP windows so it pipelines into the gather tail
                with tc.tile_pool(name="p3z", bufs=3) as p3z, \
                     tc.tile_pool(name="p3g", bufs=3) as p3g, \
                     tc.tile_pool(name="p3h", bufs=3) as p3h, \
                     tc.tile_pool(name="p3ps", bufs=1, space="PSUM") as p3ps, \
                     tc.tile_pool(name="p3psb", bufs=2, space="PSUM") as p3psb, \
                     tc.tile_pool(name="p3ps2", bufs=1, space="PSUM") as p3ps2:
                    for gi_ in range(NGRP):
                        zgrp = z_grps[gi_]
                        gw = zgrp.shape[1] // D
                        cw = gw * 128
                        zTg = p3z.tile([D + 1, GRP * 128], f32, tag="zTg")
                        nc.gpsimd.memset(zTg[D:D + 1, :cw], 1.0)
                        for k in range(gw):
                            pst = p3ps.tile([D, 128], f32, tag="pst")
                            nc.tensor.transpose(out=pst[:],
                                                in_=zgrp[:, k * D:(k + 1) * D],
                                                identity=ident[:])
                            nc.vector.tensor_copy(
                                out=zTg[:D, k * 128:(k + 1) * 128], in_=pst[:])
                        ps = p3psb.tile([D, GRP * 128], f32, tag="ps3")
                        nc.tensor.matmul(out=ps[:, :cw], lhsT=ub1[:],
                                         rhs=zTg[:, :cw], start=True, stop=True)
                        g1Tg = p3g.tile([D + 1, GRP * 128], f32, tag="g1Tg")
                        nc.scalar.activation(out=g1Tg[:D, :cw], in_=ps[:, :cw],
                                             func=Relu)
                        nc.gpsimd.memset(g1Tg[D:D + 1, :cw], 1.0)
                        h_sb = p3h.tile([128, GRP * D], f32, tag="h_sb")
                        for k in range(gw):
                            ps2 = p3ps2.tile([128, D], f32, tag="ps4")
                            nc.tensor.matmul(out=ps2[:],
                                             lhsT=g1Tg[:, k * 128:(k + 1) * 128],
                                             rhs=ub2[:], start=True, stop=True)
                            nc.vector.tensor_scalar_max(
                                out=h_sb[:, k * D:(k + 1) * D], in0=ps2[:],
                                scalar1=0.0)
                        nc.sync.dma_start(
                            out=h_out[gi_ * GRP * 128:(gi_ * GRP + gw) * 128, :]
                                .rearrange("(t p) d -> p t d", p=128),
                            in_=h_sb[:, :gw * D].rearrange("p (t d) -> p t d", d=D))

    nc.compile()
    return nc


# ------------------------------------------------------------------- kernel
def kernel(**inputs):
    from concourse.bass_utils import run_bass_kernel_spmd

    NPC, NW, NPAD, MT_ROWS = _dims()
    y = np.asarray(inputs["y"], np.float32)
    src = np.asarray(inputs["src"])
    dst = np.asarray(inputs["dst"])
    Ws = {k: np.asarray(inputs[k], np.float32)
          for k in ("W1", "b1", "W2", "b2", "U1", "c1", "U2", "c2")}

    sched, per_core = _host_schedule(src, dst)
    key = (tuple(sched["CLo"]), tuple(sched["CHi"]))
    if key not in _COMPILED:
        _COMPILED[key] = _build_program(sched)
    nc = _COMPILED[key]

    yT = np.zeros((D + 1, MT_ROWS), np.float32)
    yT[:D, :N_NODES] = y.T
    yT[D, :] = 1.0
    wb1 = np.concatenate([Ws["W1"], Ws["b1"][None, :]], axis=0)
    wb2 = np.concatenate([Ws["W2"], Ws["b2"][None, :]], axis=0)
    ub1 = np.concatenate([Ws["U1"], Ws["c1"][None, :]], axis=0)
    ub2 = np.concatenate([Ws["U2"], Ws["c2"][None, :]], axis=0)
    iota = np.tile(np.arange(128, dtype=np.float32), (128, 1))

    in_maps = []
    for c in range(NC):
        pc = per_core[c]
        in_maps.append({
            "yT": yT, "wb1": wb1, "wb2": wb2, "ub1": ub1, "ub2": ub2,
            "idx_lo": pc["idx_lo"], "idx_hi": pc["idx_hi"],
            "nodeid": pc["nodeid"], "iota128": iota,
        })

    res = run_bass_kernel_spmd(nc, in_maps, core_ids=list(range(NC)),
                               trace=bool(int(os.environ.get("KERNEL_TRACE", "0"))))
    kernel.last_results = res
    kernel.last_exec_time_ns = res.exec_time_ns

    h_full = np.zeros((N_NODES, D), np.float32)
    for c in range(NC):
        out = res.results[c]["h_out"]
        perm = per_core[c]["perm"]
        valid = perm >= 0
        h_full[perm[valid]] = out[valid]
    return h_full



# revision 10
# speedup vs baseline: 2.6804x; 2.6804x over previous
"""GNN message-passing aggregator on 8 Trainium2 NeuronCores — gatherless design.

  h = relu(relu(z @ U1 + c1) @ U2 + c2)
  z = segment_sum(relu(relu(y[src] @ W1 + b1) @ W2 + b2), dst)

Strategy (v2, replaces the dma_gather design: Pool desc-gen was 1.66ms):
  * MLP(y[src]) == MLP(y)[src]: compute m = MLP(y) once per node (phase 1),
    keep it in SBUF as bf16 [128 x NJ x 64] organized by host-chosen J-groups
    of 128 srcs (y is fed pre-permuted so no on-chip shuffle is needed).
  * Edges are sharded by dst across cores; each core's 6250 dsts form 49
    windows of 128 (greedy degree-balanced).  48 windows go through a dense
    (J, I, s<=8) bucket grid (host balances J-groups with a 32-choice greedy
    so bucket overflow is tiny); window 49 + overflow go through a small
    dma_gather sidecar (pair-indexed bf16 m table in DRAM).
  * Grid pipeline per pass (16 windows): PE expand (one matmul per J: one-hot
    Q [src x slot] x m_J -> G slots in PSUM) -> DVE copy to bf16 staging ->
    DMA to a DRAM bounce buffer in (I, J, s, f) order -> DMA back to SBUF in
    (part=(J%16,s), I, J//16, f) order -> PE scatter (lhsT=G column, rhs =
    one-hot P of dst labels) accumulating z^T [64 x 128] per window in PSUM.
  * Phase 3 consumes z^T directly (no transposes): ub1 matmul + relu + flip
    to [node x 64] via the second matmul, relu, DMA out.
  * All one-hots are built by DVE is_equal from host-streamed label arrays
    (Q labels replicated across partitions, int8 by default).
"""

import os

import numpy as np

# ---------------------------------------------------------------- constants
N_NODES = 50000
D = 64
NC = 8
NPC = N_NODES // NC          # 6250 dsts per core
NW = 49                      # dst windows per core
NWG = 48                     # windows handled by the grid
PASSES = 3
IPP = 16                     # windows per pass
NJ = 400                     # src J-groups (128 srcs each)
SCAP = 8                     # bucket capacity (J, I)
MPOS = NJ * 128              # padded node positions (51200)
NPAIR = MPOS // 2            # m_dram pair rows
COLS_I = NJ * SCAP // 128    # scatter columns per window (25)
GSLOTS = NJ * IPP * SCAP     # grid slots per pass (51200)
ICH = 4                      # windows per dma2 chunk
OP_COLS = 8                  # sidecar gather columns per op

_COMPILED = {}


def _bf16():
    import ml_dtypes
    return ml_dtypes.bfloat16


# ------------------------------------------------------------ host schedule
def _host_schedule(src, dst):
    """Per-core: I windows (degree-balanced), J groups (overflow-balanced),
    bucket grid labels, sidecar columns. Returns (sched, per_core)."""
    bf16 = _bf16()
    rng = np.random.default_rng(12345)
    src = src.astype(np.int64)
    dst = dst.astype(np.int64)

    per_core_raw = []
    scols_max = np.zeros(NW, np.int64)

    for c in range(NC):
        lo = c * NPC
        sel = (dst >= lo) & (dst < lo + NPC)
        s_e = src[sel]
        d_e = dst[sel] - lo

        # --- I windows: greedy degree balance over 6250 dsts -> 49x128
        deg = np.bincount(d_e, minlength=NPC)
        order = np.argsort(-deg, kind="stable")
        wload = np.zeros(NW, np.int64)
        wcount = np.zeros(NW, np.int64)
        asg_I = np.zeros(NPC, np.int64)
        lab_I = np.zeros(NPC, np.int64)
        for n in order:
            avail = np.flatnonzero(wcount < 128)
            w = avail[np.argmin(wload[avail])]
            asg_I[n] = w
            lab_I[n] = wcount[w]
            wcount[w] += 1
            wload[w] += deg[n]
        I_e = asg_I[d_e]

        # --- J groups: 32-choice greedy bucket balancing over all 50000 srcs
        sdeg = np.bincount(s_e, minlength=N_NODES)
        sorder = np.argsort(-sdeg, kind="stable")
        es = np.argsort(s_e, kind="stable")
        ss = s_e[es]
        ii = I_e[es]
        starts = np.searchsorted(ss, np.arange(N_NODES))
        ends = np.searchsorted(ss, np.arange(N_NODES), side="right")
        cnt = np.zeros((NJ, NWG), np.int32)
        jcount = np.zeros(NJ, np.int32)
        asg_J = np.zeros(N_NODES, np.int64)
        lab_J = np.zeros(N_NODES, np.int64)
        for n in sorder:
            a, b = starts[n], ends[n]
            Is = ii[a:b]
            Is = Is[Is < NWG]
            cands = rng.integers(0, NJ, 32)
            cands = cands[jcount[cands] < 128]
            if len(cands) == 0:
                cands = np.flatnonzero(jcount < 128)[:32]
            if len(Is):
                sub = cnt[cands][:, Is]
                ov = np.maximum(sub + 1 - SCAP, 0).sum(1)
                j = cands[np.argmin(ov)]
            else:
                j = cands[np.argmin(jcount[cands])]
            asg_J[n] = j
            lab_J[n] = jcount[j]
            jcount[j] += 1
            if len(Is):
                np.add.at(cnt, (j, Is), 1)

        # --- bucket fill: first SCAP edges -> grid, rest + w48 -> sidecar
        J_e = asg_J[s_e]
        qlab = np.full((PASSES, NJ, SCAP, IPP), -1, np.int64)
        plab = np.full((PASSES, IPP, COLS_I, 128), -1, np.int64)
        fill = np.zeros((NJ, NWG), np.int64)
        side = [[] for _ in range(NW)]     # (pos, dstlab) per window
        pos_e = asg_J[s_e] * 128 + lab_J[s_e]
        dl_e = lab_I[d_e]
        for k in range(len(s_e)):
            I = I_e[k]
            if I < NWG:
                J = J_e[k]
                f = fill[J, I]
                if f < SCAP:
                    fill[J, I] = f + 1
                    p, Il = I // IPP, I % IPP
                    qlab[p, J, f, Il] = lab_J[s_e[k]]
                    # scatter col: q = (J%16)*8+s, c = J//16
                    plab[p, Il, J // 16, (J % 16) * SCAP + f] = dl_e[k]
                else:
                    side[I].append((pos_e[k], dl_e[k]))
            else:
                side[I].append((pos_e[k], dl_e[k]))

        scols = np.array([(len(side[I]) + 127) // 128 for I in range(NW)])
        scols_max = np.maximum(scols_max, scols)
        per_core_raw.append((qlab, plab, side, asg_J, lab_J, asg_I, lab_I))

    scols_max = np.maximum(scols_max, 1)   # >=1 col per window for shape unif.
    stot = int(scols_max.sum())
    sched = {"scols": scols_max.astype(int).tolist(), "stot": stot}

    qlab_i8 = not bool(int(os.environ.get("KERNEL_QLAB_BF16", "0")))
    per_core = []
    for c in range(NC):
        qlab, plab, side, asg_J, lab_J, asg_I, lab_I = per_core_raw[c]
        # qlab stream: [128 x PASSES*NJ*128] replicated across partitions
        q_flat = qlab.reshape(PASSES, NJ, SCAP * IPP).reshape(-1)
        qd = np.int8 if qlab_i8 else bf16
        qlab_rep = np.ascontiguousarray(
            np.tile(q_flat[None, :], (128, 1)).astype(qd))
        # plab stream: [128 x PASSES*IPP*COLS_I] column-major labels
        plab_2d = np.ascontiguousarray(
            plab.reshape(PASSES * IPP * COLS_I, 128).T.astype(bf16))

        # sidecar: pad each window to scols_max[c] columns
        sc_idx = np.zeros((stot * 128,), np.int64)
        sc_lab = np.full((2, stot * 128), -1, np.int64)   # even/odd labels
        base = 0
        for I in range(NW):
            lst = side[I]
            for k, (pos, dlab) in enumerate(lst):
                sc_idx[base * 128 + k] = pos // 2
                sc_lab[pos % 2, base * 128 + k] = dlab
            base += int(sched["scols"][I])
        # wrap idx into per-op [16, ni/16] layout replicated to 128 partitions
        blocks = []
        posn = 0
        while posn < stot:
            k = min(OP_COLS, stot - posn)
            op = sc_idx[posn * 128:(posn + k) * 128].astype(np.int16)
            blk = op.reshape(k * 128 // 16, 16).T
            blocks.append(np.tile(blk, (8, 1)))
            posn += k
        scidx = np.ascontiguousarray(np.concatenate(blocks, axis=1))
        sclab = np.ascontiguousarray(
            sc_lab.reshape(2, stot, 128).transpose(2, 1, 0).reshape(128, stot * 2)
            .astype(bf16))
        # ^ [128 part x (col, parity)] : col-major pairs (even, odd)

        # permuted y^T (bf16) and output perm
        yT = np.zeros((D + 1, MPOS), np.float32)
        perm_m = np.full(MPOS, -1, np.int64)
        perm_m[asg_J * 128 + lab_J] = np.arange(N_NODES)
        valid = perm_m >= 0
        per_core.append({"perm_m": perm_m, "valid": valid})
        yTv = per_core[-1]

        perm_h = np.full(NW * 128, -1, np.int64)
        perm_h[asg_I * 128 + lab_I] = np.arange(NPC) + c * NPC

        yTv.update({
            "qlab": qlab_rep, "plab": plab_2d,
            "scidx": scidx, "sclab": sclab, "perm_h": perm_h,
        })
    sched["qlab_i8"] = qlab_i8
    return sched, per_core, per_core_raw


# ------------------------------------------------------------- bass program
def _build_program(sched):
    import concourse.bacc as bacc
    import concourse.mybir as mybir
    import concourse.tile as tile
    from concourse.tile import add_dep_helper

    f32 = mybir.dt.float32
    bf = mybir.dt.bfloat16
    i16 = mybir.dt.int16
    i8 = mybir.dt.int8
    Relu = mybir.ActivationFunctionType.Relu
    Copy = mybir.ActivationFunctionType.Copy

    scols = sched["scols"]
    stot = sched["stot"]
    qdt = i8 if sched["qlab_i8"] else bf
    CHUNK = 512
    NCH = MPOS // CHUNK                    # phase-1 chunks (100)
    QCH = 5120                             # qlab chunk: 5 slabs (5x1024), 30 total
    NQCH = PASSES * NJ * 128 // QCH
    assert NQCH * QCH == PASSES * NJ * 128 and QCH % 1024 == 0

    nc = bacc.Bacc()
    yT_in = nc.dram_tensor("yT", [D + 1, MPOS], bf, kind="ExternalInput")
    wb1_in = nc.dram_tensor("wb1", [D + 1, D], bf, kind="ExternalInput")
    wb2_in = nc.dram_tensor("wb2", [D + 1, D], bf, kind="ExternalInput")
    ub1_in = nc.dram_tensor("ub1", [D + 1, D], bf, kind="ExternalInput")
    ub2_in = nc.dram_tensor("ub2", [D + 1, D], bf, kind="ExternalInput")
    qlab_in = nc.dram_tensor("qlab", [128, PASSES * NJ * 128], qdt,
                             kind="ExternalInput")
    plab_in = nc.dram_tensor("plab", [128, PASSES * IPP * COLS_I], bf,
                             kind="ExternalInput")
    iota_in = nc.dram_tensor("iota128", [128, 128], bf, kind="ExternalInput")
    iotap_in = nc.dram_tensor("iotaP", [128, 1], qdt, kind="ExternalInput")
    scidx_in = nc.dram_tensor("scidx", [128, stot * 8], i16, kind="ExternalInput")
    sclab_in = nc.dram_tensor("sclab", [128, stot * 2], bf, kind="ExternalInput")
    m_dram = nc.dram_tensor("m_pairs", [NPAIR, 128], bf, kind="Internal")
    gdram = [nc.dram_tensor(f"gbounce{p}", [16, SCAP, IPP, COLS_I, D], bf,
                            kind="Internal")
             for p in range(PASSES)]
    h_out = nc.dram_tensor("h_out", [NW * 128, D], f32, kind="ExternalOutput")

    with tile.TileContext(nc) as tc:
        with tc.tile_pool(name="const", bufs=1) as cpool, \
             tc.tile_pool(name="mtab", bufs=1) as mpool, \
             tc.tile_pool(name="scg", bufs=1) as scgp:
            wb1 = cpool.tile([D + 1, D], bf, tag="wb1")
            wb2 = cpool.tile([D + 1, D], bf, tag="wb2")
            ub1 = cpool.tile([D + 1, D], bf, tag="ub1")
            ub2 = cpool.tile([D + 1, D], bf, tag="ub2")
            iota = cpool.tile([128, 128], bf, tag="iota")
            iotap = cpool.tile([128, 1], qdt, tag="iotap")
            plab_t = cpool.tile([128, PASSES * IPP * COLS_I], bf, tag="plab")
            sclab_t = cpool.tile([128, stot * 2], bf, tag="sclab")
            scidx_t = cpool.tile([128, stot * 8], i16, tag="scidx")
            nc.sync.dma_start(out=wb1[:], in_=wb1_in[:])
            nc.sync.dma_start(out=wb2[:], in_=wb2_in[:])
            nc.sync.dma_start(out=ub1[:], in_=ub1_in[:])
            nc.sync.dma_start(out=ub2[:], in_=ub2_in[:])
            nc.sync.dma_start(out=iota[:], in_=iota_in[:])
            nc.sync.dma_start(out=iotap[:], in_=iotap_in[:])
            nc.sync.dma_start(out=plab_t[:], in_=plab_in[:])
            nc.sync.dma_start(out=sclab_t[:], in_=sclab_in[:])
            nc.sync.dma_start(out=scidx_t[:], in_=scidx_in[:])

            m_sb = mpool.tile([128, NJ, D], bf, tag="m_sb")

            # ---------------- phase 1: m = MLP1(y) -> m_sb (bf16) ------------
            with tc.tile_pool(name="p1y", bufs=3) as p1y, \
                 tc.tile_pool(name="p1h", bufs=1) as p1h, \
                 tc.tile_pool(name="p1ps", bufs=2, space="PSUM") as p1ps, \
                 tc.tile_pool(name="p1ps2", bufs=2, space="PSUM") as p1ps2:
                h1a = p1h.tile([D + 1, CHUNK], bf, tag="h1a")
                h1b = p1h.tile([D + 1, CHUNK], bf, tag="h1b")
                nc.gpsimd.memset(h1a[D:D + 1, :], 1.0)
                nc.gpsimd.memset(h1b[D:D + 1, :], 1.0)
                h1bufs = [h1a, h1b]
                for ch in range(NCH):
                    c0 = ch * CHUNK
                    ytile = p1y.tile([D + 1, CHUNK], bf, tag="ytile")
                    nc.scalar.dma_start(out=ytile[:], in_=yT_in[:, c0:c0 + CHUNK])
                    ps = p1ps.tile([D, CHUNK], f32, tag="ps1")
                    nc.tensor.matmul(out=ps[:], lhsT=wb1[:], rhs=ytile[:],
                                     start=True, stop=True)
                    h1c = h1bufs[ch % 2]
                    nc.scalar.activation(out=h1c[:D, :], in_=ps[:], func=Relu)
                    ps2 = p1ps2.tile([128, 4 * D], f32, tag="ps2")
                    for k in range(4):
                        nc.tensor.matmul(out=ps2[:, k * D:(k + 1) * D],
                                         lhsT=h1c[:, k * 128:(k + 1) * 128],
                                         rhs=wb2[:], start=True, stop=True)
                    nc.vector.tensor_scalar_max(
                        out=m_sb[:, ch * 4:(ch + 1) * 4, :],
                        in0=ps2[:].rearrange("p (t d) -> p t d", d=D),
                        scalar1=0.0)

            # m pair table for the sidecar gathers: [NPAIR, 128] bf16
            wr_m = nc.sync.dma_start(
                out=m_dram[:].rearrange("(j kh) (par f) -> (kh par) j f",
                                        j=NJ, kh=64, par=2, f=D),
                in_=m_sb[:])
            jm = nc.sync.nop(nofuse=True)
            add_dep_helper(jm.ins, wr_m.ins, sync=True, reason="m pairs ready")

            # sidecar gathers (Pool) — issue early, they overlap the grid
            g_tiles = {}
            with tc.tile_pool(name="sgat", bufs=1) as sgp:
                posn = 0
                oi = 0
                while posn < stot:
                    k = min(OP_COLS, stot - posn)
                    g = sgp.tile([128, k, 128], bf, tag=f"g{oi}")
                    ni = k * 128
                    gi = nc.gpsimd.dma_gather(
                        out_ap=g[:], in_ap=m_dram[:],
                        idxs_ap=scidx_t[:, posn * 8:posn * 8 + k * 8],
                        num_idxs=ni, num_idxs_reg=ni, elem_size=128)
                    add_dep_helper(gi.ins, jm.ins, sync=True,
                                   reason="gather after m ready")
                    for kk in range(k):
                        g_tiles[posn + kk] = (g, kk)
                    posn += k
                    oi += 1

                # ---------------- phase 2 + 3 ------------------------------
                scol_off = np.concatenate([[0], np.cumsum(scols)]).astype(int)
                with tc.tile_pool(name="qstr", bufs=2) as qstr, \
                     tc.tile_pool(name="qoh", bufs=2) as qoh, \
                     tc.tile_pool(name="poh", bufs=2) as poh, \
                     tc.tile_pool(name="stg", bufs=3) as stgp, \
                     tc.tile_pool(name="rbuf", bufs=5) as rbp, \
                     tc.tile_pool(name="pse", bufs=2, space="PSUM") as psep, \
                     tc.tile_pool(name="zt", bufs=4, space="PSUM") as ztp, \
                     tc.tile_pool(name="p3a", bufs=1, space="PSUM") as p3a, \
                     tc.tile_pool(name="p3b", bufs=1, space="PSUM") as p3b, \
                     tc.tile_pool(name="p3s", bufs=2) as p3s, \
                     tc.tile_pool(name="p3g", bufs=2) as p3g, \
                     tc.tile_pool(name="p3h", bufs=2) as p3h:

                    # qlab chunks: QCH slots each, issued just-in-time
                    qstream = {}

                    def ensure_qchunk(qc):
                        if qc not in qstream and qc < NQCH:
                            qt = qstr.tile([128, QCH], qdt, tag="qs")
                            nc.scalar.dma_start(
                                out=qt[:],
                                in_=qlab_in[:, qc * QCH:(qc + 1) * QCH])
                            qstream[qc] = qt
                        return qstream.get(qc)

                    def build_q(slot0, width, pool):
                        """one-hot Q [128 x width] bf16 for grid slots
                        [slot0, slot0+width) (within one qlab chunk)."""
                        qc, off = slot0 // QCH, slot0 % QCH
                        qt = ensure_qchunk(qc)
                        ensure_qchunk(qc + 1)
                        q = pool.tile([128, width], bf, tag="qoh")
                        nc.vector.tensor_tensor(
                            out=q[:], in0=qt[:, off:off + width],
                            in1=iotap[:].to_broadcast([128, width]),
                            op=mybir.AluOpType.is_equal)
                        return q

                    dma1s = [[] for _ in range(PASSES)]
                    dma2s = [[] for _ in range(PASSES)]
                    zt_tiles = {}
                    h_grp = []

                    def expand_pass(p):
                        for slab in range(NJ // 8):          # 8 J per slab
                            J0 = slab * 8
                            s0 = (p * NJ + J0) * 128
                            q8 = build_q(s0, 8 * 128, qoh)
                            pse = psep.tile([128, 8 * D], f32, tag="pse")
                            for j in range(8):
                                nc.tensor.matmul(
                                    out=pse[:, j * D:(j + 1) * D],
                                    lhsT=q8[:, j * 128:(j + 1) * 128],
                                    rhs=m_sb[:, J0 + j, :],
                                    start=True, stop=True)
                            stg = stgp.tile([128, 8 * D], bf, tag="stg")
                            nc.vector.tensor_copy(out=stg[:], in_=pse[:])
                            cblk, jlb = J0 // 16, J0 % 16
                            d1 = nc.sync.dma_start(
                                out=gdram[p][jlb:jlb + 8, :, :, cblk, :]
                                    .rearrange("jj ss i f -> (ss i) jj f"),
                                in_=stg[:].rearrange("p (j f) -> p j f", f=D))
                            dma1s[p].append(d1)

                    def scatter_pass(p):
                        # dma2 chunks (ICH windows each), then scatter+ph3
                        for cidx in range(IPP // ICH):
                            rb = rbp.tile([128, ICH, COLS_I, D], bf, tag="rb")
                            d2 = nc.scalar.dma_start(
                                out=rb[:].rearrange("q i c f -> q i (c f)"),
                                in_=gdram[p][:, :, cidx * ICH:(cidx + 1) * ICH]
                                    .rearrange("jj ss i c f -> (jj ss) i (c f)"))
                            for d1 in dma1s[p]:
                                add_dep_helper(d2.ins, d1.ins, sync=True,
                                               reason="bounce RAW")
                            dma2s[p].append((rb, d2))
                        zt4 = None
                        for Il in range(IPP):
                            I = p * IPP + Il
                            rb, _ = dma2s[p][Il // ICH]
                            ilc = Il % ICH
                            if Il % 4 == 0:
                                zt4 = ztp.tile([D, 4, 128], f32, tag="zt")
                            zt = zt4[:, Il % 4, :]
                            zt_tiles[I] = zt
                            # batched P one-hots for this window (25 cols)
                            pc0 = (p * IPP + Il) * COLS_I
                            poh_t = poh.tile([128, COLS_I, 128], bf, tag="poh")
                            nc.vector.tensor_tensor(
                                out=poh_t[:],
                                in0=plab_t[:, pc0:pc0 + COLS_I, None]
                                    .to_broadcast([128, COLS_I, 128]),
                                in1=iota[:, None, :]
                                    .to_broadcast([128, COLS_I, 128]),
                                op=mybir.AluOpType.is_equal)
                            nsc = scols[I]
                            total = COLS_I + 2 * nsc
                            ci = 0
                            for cc in range(COLS_I):
                                nc.tensor.matmul(
                                    out=zt[:],
                                    lhsT=rb[:, ilc, cc, :],
                                    rhs=poh_t[:, cc, :],
                                    start=(ci == 0), stop=(ci == total - 1),
                                    skip_group_check=True)
                                ci += 1
                            ci = _sidecar(I, zt, ci, total)
                            if Il % 4 == 3:
                                phase3(p * IPP + Il - 3, 4)

                    def _sidecar(I, zt, ci, total):
                        for k in range(scols[I]):
                            col = int(scol_off[I]) + k
                            g, kk = g_tiles[col]
                            pv = poh.tile([128, 2, 128], bf, tag="pscol")
                            nc.vector.tensor_tensor(
                                out=pv[:],
                                in0=sclab_t[:, col * 2:col * 2 + 2, None]
                                    .to_broadcast([128, 2, 128]),
                                in1=iota[:, None, :].to_broadcast([128, 2, 128]),
                                op=mybir.AluOpType.is_equal)
                            for par in range(2):
                                nc.tensor.matmul(
                                    out=zt[:],
                                    lhsT=g[:, kk, par * D:(par + 1) * D],
                                    rhs=pv[:, par, :],
                                    start=(ci == 0), stop=(ci == total - 1),
                                    skip_group_check=True)
                                ci += 1
                        return ci

                    def phase3(I0, gw):
                        zt1 = p3s.tile([D + 1, 4 * 128], bf, tag="zt1")
                        nc.gpsimd.memset(zt1[D:D + 1, :gw * 128], 1.0)
                        for g in range(gw):
                            nc.scalar.copy(
                                out=zt1[:D, g * 128:(g + 1) * 128],
                                in_=zt_tiles[I0 + g])
                        psA = p3a.tile([D, 4 * 128], f32, tag="psA")
                        nc.tensor.matmul(out=psA[:, :gw * 128], lhsT=ub1[:],
                                         rhs=zt1[:, :gw * 128],
                                         start=True, stop=True)
                        g1 = p3g.tile([D + 1, 4 * 128], bf, tag="g1")
                        nc.scalar.activation(out=g1[:D, :gw * 128],
                                             in_=psA[:, :gw * 128], func=Relu)
                        nc.gpsimd.memset(g1[D:D + 1, :gw * 128], 1.0)
                        psB = p3b.tile([128, 4 * D], f32, tag="psB")
                        h_sb = p3h.tile([128, 4 * D], f32, tag="h_sb")
                        for g in range(gw):
                            nc.tensor.matmul(out=psB[:, g * D:(g + 1) * D],
                                             lhsT=g1[:, g * 128:(g + 1) * 128],
                                             rhs=ub2[:], start=True, stop=True)
                        nc.vector.tensor_scalar_max(
                            out=h_sb[:, :gw * D], in0=psB[:, :gw * D],
                            scalar1=0.0)
                        nc.sync.dma_start(
                            out=h_out[I0 * 128:(I0 + gw) * 128, :]
                                .rearrange("(t p) d -> p t d", p=128),
                            in_=h_sb[:, :gw * D].rearrange("p (t d) -> p t d",
                                                           d=D))

                    # schedule: exp0, exp1, scat0, exp2, scat1, scat2, w48
                    expand_pass(0)
                    expand_pass(1)
                    scatter_pass(0)
                    expand_pass(2)
                    scatter_pass(1)
                    scatter_pass(2)
                    # window 48: sidecar only
                    I = NWG
                    zt48 = ztp.tile([D, 4, 128], f32, tag="zt")
                    zt = zt48[:, 0, :]
                    zt_tiles[I] = zt
                    nsc = scols[I]
                    ci = _sidecar(I, zt, 0, 2 * nsc)
                    phase3(NWG, 1)

    nc.compile()
    return nc


# ------------------------------------------------------------------- kernel
def kernel(**inputs):
    from concourse.bass_utils import run_bass_kernel_spmd

    bf16 = _bf16()
    y = np.asarray(inputs["y"], np.float32)
    src = np.asarray(inputs["src"])
    dst = np.asarray(inputs["dst"])
    Ws = {k: np.asarray(inputs[k], np.float32)
          for k in ("W1", "b1", "W2", "b2", "U1", "c1", "U2", "c2")}

    sched, per_core, _raw = _host_schedule(src, dst)
    key = (sched["stot"], tuple(sched["scols"]), sched["qlab_i8"])
    if key not in _COMPILED:
        _COMPILED[key] = _build_program(sched)
    nc = _COMPILED[key]

    wb1 = np.concatenate([Ws["W1"], Ws["b1"][None, :]], 0).astype(bf16)
    wb2 = np.concatenate([Ws["W2"], Ws["b2"][None, :]], 0).astype(bf16)
    ub1 = np.concatenate([Ws["U1"], Ws["c1"][None, :]], 0).astype(bf16)
    ub2 = np.concatenate([Ws["U2"], Ws["c2"][None, :]], 0).astype(bf16)
    iota128 = np.tile(np.arange(128, dtype=np.float32), (128, 1)).astype(bf16)
    qdt = np.int8 if sched["qlab_i8"] else bf16
    iotaP = np.arange(128, dtype=np.float32)[:, None].astype(qdt)

    in_maps = []
    for c in range(NC):
        pc = per_core[c]
        yT = np.zeros((D + 1, MPOS), np.float32)
        valid = pc["valid"]
        yT[:D, valid] = y[pc["perm_m"][valid]].T
        yT[D, :] = 1.0
        in_maps.append({
            "yT": yT.astype(bf16), "wb1": wb1, "wb2": wb2, "ub1": ub1,
            "ub2": ub2, "qlab": pc["qlab"], "plab": pc["plab"],
            "iota128": iota128, "iotaP": iotaP, "scidx": pc["scidx"],
            "sclab": pc["sclab"],
        })

    res = run_bass_kernel_spmd(nc, in_maps, core_ids=list(range(NC)),
                               trace=bool(int(os.environ.get("KERNEL_TRACE", "0"))))
    kernel.last_results = res
    kernel.last_exec_time_ns = res.exec_time_ns

    h_full = np.zeros((N_NODES, D), np.float32)
    for c in range(NC):
        out = res.results[c]["h_out"]
        perm_h = per_core[c]["perm_h"]
        valid = perm_h >= 0
        h_full[perm_h[valid]] = out[valid]
    return h_full
